# revision 2
# baseline (speedup 1.0000x reference)
"""Causal multi-head attention (B=4, H=16, S=2048, D=64) on 8 TRN2 NeuronCores.

Sharding: B*H = 64 heads, 8 heads per core (data/head parallel, no comms).

v2 pipeline (per head) -- dual-engine exp + DMA transposes:
  - DMA Q,K,V [2048,64] f32 -> SBUF; cast to bf16 on GpSimd into a
    d-padded [128, 16, 128] layout (cols 64:128 zeroed once per pool slot)
  - Q,K transposed to d-major via batched SBUF->SBUF xbar DMA transposes
    ([128,t,128] slabs); the K=128 contraction pad falls out of
    transposing the padded input.  No PE transposes, no PSUM staging
    bank, no DVE flatten copies.
  - stream layout per head: 16 causal-diagonal 128x128 tiles FIRST, then
    the below-diagonal strips (strip j covers q >= 128(j+1)).  All 16
    diagonal trimask multiplies batch into ONE broadcast tensor_tensor.
  - QK^T emitted as 512-col chunks into a rotating 2x3-bank PSUM ring.
  - exp SPLIT across two engines: ScalarE ACTIVATE (exact, ~21/34
    chunks) and VectorE (~13/34 chunks) via the Schraudolph bf16 bit
    trick -- one tensor_scalar: i16 = round(x*SCALE*128/ln2 + 16248.5)
    written through an int16 bitcast of the bf16 at_all buffer (max rel
    err ~4%, mean-centered; post-softmax contribution ~5e-3).
  - A@V with a ones-column appended to V for the softmax denominator;
    o4 PSUM accumulator double-buffered (bank freed by dropping the
    transpose staging); normalize via DVE reciprocal + multiply.
  - head 0 runs tiles 8-15 first (needs only the second DMA split); the
    next head's first chunks are pre-emitted before the current head's
    last group so the exp stream crosses head boundaries without a gap.

PSUM budget: exp ring 2x3 banks + A@V accum 2x1 = 8.
"""

import os
import sys

try:
    import concourse.bass as bass  # noqa: F401
except ImportError:
    sys.path.insert(0, "/opt/trn_rl_repo")

import numpy as np

import concourse.mybir as mybir
import concourse.tile as tile
from concourse import bacc
from concourse.bass_utils import run_bass_kernel_spmd

B, H, S, D = 4, 16, 2048, 64
N_CORES = 8
HEADS = B * H
HPC = HEADS // N_CORES  # heads per core
P = 128
ST = S // P  # 16 s-tiles per head

F32 = mybir.dt.float32
BF16 = mybir.dt.bfloat16
I16 = mybir.dt.int16

SCALE = 1.0 / float(np.sqrt(D))
SCHRAUD_A = 128.0 / float(np.log(2.0))   # bf16 exponent scale
SCHRAUD_B = 16256.0 - 7.5                # bias, mean-centering correction

CH = 512                       # stream chunk width (one PSUM bank)
GRP = int(os.environ.get("K_GRP", "3"))   # chunks per exp group
LAG = int(os.environ.get("K_LAG", "1"))   # A@V lag in groups
CAP = int(os.environ.get("K_CAP", "10"))  # max A@V matmuls scheduled per group
PREP_G = int(os.environ.get("K_PREPG", "4"))  # group idx to start next head's prep
# groups whose exp runs on the Vector engine (Schraudolph); rest on ScalarE
VGROUPS = set(int(g) for g in os.environ.get("K_VG", "2,4,6,8,10").split(",") if g)
CAST_ENG = os.environ.get("K_CAST_ENG", "gpsimd")   # gpsimd | vector
MASK_ENG = os.environ.get("K_MASK_ENG", "vector")   # gpsimd | vector

# ---- stream geometry ------------------------------------------------------
# segment = ("d", j) diagonal tile of strip j (width 128, q in [128j,128j+128))
#         = ("o", j) off-diag strip j (width 1920-128j, q in [128(j+1), 2048))
W_OFF = [S - P * (j + 1) for j in range(ST)]  # off width per strip (j=15 -> 0)

SEGS = {
    "n": [("d", j) for j in range(ST)] + [("o", j) for j in range(ST - 1)],
    "h0": ([("d", j) for j in range(8, ST)] + [("o", j) for j in range(8, ST - 1)]
           + [("d", j) for j in range(8)] + [("o", j) for j in range(8)]),
}
STREAM = ST * P + sum(W_OFF)                  # 17408
NCH = STREAM // CH                            # 34 chunks per head

DIAG_OFF = {}
OFF_OFF = {}
for _kind, _segs in SEGS.items():
    dd, oo, pos = {}, {}, 0
    for (typ, j) in _segs:
        if typ == "d":
            dd[j] = pos
            pos += P
        else:
            oo[j] = pos
            pos += W_OFF[j]
    DIAG_OFF[_kind] = dd
    OFF_OFF[_kind] = oo

# exp groups per head; mid heads put the short group second-to-last (the
# next head's first chunks WAR on the ring released by that group's exp).
GROUPS = {"last": []}
_c = 0
while _c < NCH:
    GROUPS["last"].append((_c, min(_c + GRP, NCH)))
    _c += GRP
GROUPS["mid"] = list(GROUPS["last"])
if len(GROUPS["mid"]) >= 2 and NCH % GRP:
    _a, _b = GROUPS["mid"][-2]
    _rem = NCH % GRP
    GROUPS["mid"][-2] = (_a, _a + _rem)
    GROUPS["mid"][-1] = (_a + _rem, NCH)
NGRP = len(GROUPS["mid"])
GROUP_OF_CHUNK = {}
for _gk, _ranges in GROUPS.items():
    m = {}
    for _g, (_a, _b) in enumerate(_ranges):
        for _cc in range(_a, _b):
            m[_cc] = _g
    GROUP_OF_CHUNK[_gk] = m

# mask ops: (gate_chunk, at_off, ntiles) -- one broadcast trimask multiply
# over ntiles diagonal tiles starting at stream offset at_off, emitted after
# the group holding gate_chunk has exp'd.
MASK_OPS = {
    "n": [((ST * P - 1) // CH, 0, ST)],
    "h0": [((8 * P - 1) // CH, 0, 8),
           (((DIAG_OFF["h0"][0] + 8 * P) - 1) // CH, DIAG_OFF["h0"][0], 8)],
}


def head_kind(h):
    return "h0" if h == 0 else "n"


def chunk_pieces(kind, c):
    """Matmul pieces of stream chunk c: list of (strip j, q0, ring_col, w)."""
    lo, hi = CH * c, CH * (c + 1)
    out = []
    pos = 0
    for (typ, j) in SEGS[kind]:
        w = P if typ == "d" else W_OFF[j]
        a, b = max(lo, pos), min(hi, pos + w)
        if a < b:
            q0 = (P * j if typ == "d" else P * (j + 1)) + (a - pos)
            out.append((j, q0, a - lo, b - a))
        pos += w
    return out


def av_regions(kind, jq):
    """at_all offsets of the (jq+1) lhsT tiles for A@V of q-tile jq."""
    regs = [DIAG_OFF[kind][jq]]
    for k in range(jq):
        regs.append(OFF_OFF[kind][k] + P * (jq - k - 1))
    return regs


def ready_group(kind, gk, jq):
    """Exp group after which A@V for q-tile jq can run: all lhsT regions
    exp'd and the covering mask op's gate group passed."""
    pos = max(r + P for r in av_regions(kind, jq))
    g = GROUP_OF_CHUNK[gk][(pos - 1) // CH]
    for (gate_c, off, ntiles) in MASK_OPS[kind]:
        if off <= DIAG_OFF[kind][jq] < off + ntiles * P:
            g = max(g, GROUP_OF_CHUNK[gk][gate_c])
    return g


def build_nc(heads_per_core=HPC):
    nc = bacc.Bacc("TRN2", target_bir_lowering=False, debug=False,
                   num_devices=N_CORES)
    q_d = nc.dram_tensor("Q", [heads_per_core, S, D], F32, kind="ExternalInput")
    k_d = nc.dram_tensor("K", [heads_per_core, S, D], F32, kind="ExternalInput")
    v_d = nc.dram_tensor("V", [heads_per_core, S, D], F32, kind="ExternalInput")
    o_d = nc.dram_tensor("out", [heads_per_core, S, D], F32, kind="ExternalOutput")

    cast_eng = {"gpsimd": (lambda: nc.gpsimd), "vector": (lambda: nc.vector)}[CAST_ENG]
    mask_eng = {"gpsimd": (lambda: nc.gpsimd), "vector": (lambda: nc.vector)}[MASK_ENG]

    with tile.TileContext(nc) as tc:
        with (
            tc.tile_pool(name="const", bufs=1) as const,
            tc.tile_pool(name="stage", bufs=2) as stage,
            tc.tile_pool(name="bfp", bufs=2) as bfp,
            tc.tile_pool(name="tp", bufs=2) as tpool,
            tc.tile_pool(name="atp", bufs=2) as atp,
            tc.tile_pool(name="osb", bufs=2) as osbp,
            tc.tile_pool(name="small", bufs=8) as small,
            tc.tile_pool(name="ps", bufs=1, space="PSUM") as ps,
        ):
            # upper-triangular (incl. diagonal) ones: keep q >= k
            trimask = const.tile([P, P], BF16, tag="trimask")
            nc.gpsimd.memset(trimask, 1.0)
            nc.gpsimd.affine_select(
                out=trimask, in_=trimask,
                compare_op=mybir.AluOpType.is_ge,
                fill=0.0, base=0,
                pattern=[[1, P]], channel_multiplier=-1,
            )

            def emit_prep(h):
                """Load + cast + DMA-transpose head h's operands."""
                q_raw = stage.tile([P, ST, D], F32, tag="qraw")
                k_raw = stage.tile([P, ST, D], F32, tag="kraw")
                v_raw = stage.tile([P, ST, D], F32, tag="vraw")
                # d-padded bf16 copies: cols 64:128 stay zero across heads
                q_bf = bfp.tile([P, ST, P], BF16, tag="qbf")
                k_bf = bfp.tile([P, ST, P], BF16, tag="kbf")
                qT3 = tpool.tile([P, ST, P], BF16, tag="qT3")
                kT3 = tpool.tile([P, ST, P], BF16, tag="kT3")
                if h < 2:  # pool slots keep their zero pad across heads
                    nc.gpsimd.memset(q_bf[:, :, D:P], 0.0)
                    nc.gpsimd.memset(k_bf[:, :, D:P], 0.0)
                if h == 0:
                    # tiles 8-15 load/cast/transpose first: the only deps of
                    # exp groups 0-2, so the first exp starts after half the
                    # DMA.  0-7 follows as the tail after group 2.
                    splits = [(8, 12), (12, 16), (0, 8)]
                else:
                    splits = [(0, 16)]
                for s0, s1 in splits:
                    for (raw, d_) in ((q_raw, q_d), (k_raw, k_d)):
                        nc.sync.dma_start(
                            out=raw[:, s0:s1, :],
                            in_=d_[h].rearrange("(b p) d -> p b d", p=P)[:, s0:s1, :])

                def emit_chain(s0, s1):
                    for (raw, bf_, t3) in ((q_raw, q_bf, qT3), (k_raw, k_bf, kT3)):
                        cast_eng().tensor_copy(bf_[:, s0:s1, 0:D], raw[:, s0:s1, :])
                        nc.sync.dma_start_transpose(
                            out=t3[:, s0:s1, :], in_=bf_[:, s0:s1, :])

                def emit_tail(lo_splits):
                    for (s0, s1) in lo_splits:
                        emit_chain(s0, s1)
                    cast_eng().tensor_copy(v_aug[:, :, 0:D], v_raw)
                    cast_eng().memset(v_aug[:, :, D:D + 1], 1.0)

                v_aug = bfp.tile([P, ST, D + 1], BF16, tag="vaug")
                nc.sync.dma_start(
                    out=v_raw, in_=v_d[h].rearrange("(b p) d -> p b d", p=P))
                if h == 0:
                    for (s0, s1) in splits[:2]:
                        emit_chain(s0, s1)
                    tail = lambda: emit_tail(splits[2:])  # noqa: E731
                else:
                    emit_tail(splits)
                    tail = None
                return (qT3.rearrange("p t c -> p (t c)"),
                        kT3.rearrange("p t c -> p (t c)"), v_aug, tail)

            # Per-head pipeline state; two heads live at once.
            state = {}

            def alloc_ring(h, g):
                return ps.tile([P, GRP, CH], F32, tag="ring", bufs=2,
                               name=f"ring_{h}_{g}")

            def group_kind(h):
                return "last" if h == heads_per_core - 1 else "mid"

            def fill_ring(h, g, ring):
                st = state[h]
                kind = head_kind(h)
                qT, kT = st["qT"], st["kT"]
                c0, c1 = GROUPS[group_kind(h)][g]
                for c in range(c0, c1):
                    for (j, qg, rcol, w) in chunk_pieces(kind, c):
                        nc.tensor.matmul(
                            ring[:, c - c0, rcol:rcol + w],
                            lhsT=kT[:, P * j:P * (j + 1)],
                            rhs=qT[:, qg:qg + w],
                            start=True, stop=True,
                        )

            def emit_exp(h, g, ring):
                st = state[h]
                at_all = st["at"]
                c0, c1 = GROUPS[group_kind(h)][g]
                nch = c1 - c0
                src = ring[:, 0:nch, :].rearrange("p a b -> p (a b)")
                if g in VGROUPS:
                    nc.vector.tensor_scalar(
                        out=at_all[:, CH * c0:CH * c1].bitcast(I16),
                        in0=src,
                        scalar1=SCALE * SCHRAUD_A, scalar2=SCHRAUD_B,
                        op0=mybir.AluOpType.mult, op1=mybir.AluOpType.add,
                    )
                else:
                    nc.scalar.activation(
                        at_all[:, CH * c0:CH * c1], src,
                        mybir.ActivationFunctionType.Exp,
                        scale=SCALE,
                    )
                # batched causal masks whose gate chunk this group covers
                kind = head_kind(h)
                for (gate_c, off, ntiles) in MASK_OPS[kind]:
                    if c0 <= gate_c < c1:
                        reg = bass.AP(
                            tensor=at_all.tensor,
                            offset=at_all.offset + off,
                            ap=[at_all.ap[0], [P, ntiles], [1, P]],
                        )
                        tm_b = bass.AP(
                            tensor=trimask.tensor, offset=trimask.offset,
                            ap=[trimask.ap[0], [0, ntiles], trimask.ap[1]],
                        )
                        mask_eng().tensor_tensor(
                            out=reg, in0=reg, in1=tm_b,
                            op=mybir.AluOpType.mult,
                        )

            def emit_group(h, g):
                ring = alloc_ring(h, g)
                fill_ring(h, g, ring)
                emit_exp(h, g, ring)

            def emit_av(h, jq):
                """A@V for q-tile jq of head h; groups of four q-tiles share
                one PSUM bank + one batched normalize; stream output DMA."""
                st = state[h]
                at_all, v_aug, o_sb = st["at"], st["v_aug"], st["o_sb"]
                kind = head_kind(h)
                if jq % 4 == 0:
                    st["o4"] = ps.tile([P, 4, D + 1], F32, tag="o",
                                       bufs=2, name="o4")
                o4 = st["o4"]
                regs = av_regions(kind, jq)  # [diag, off_0.., off_{jq-1}]
                for k in range(jq + 1):
                    a0 = regs[0] if k == jq else regs[k + 1]
                    nc.tensor.matmul(
                        o4[:, jq % 4, :],
                        lhsT=at_all[:, a0:a0 + P],
                        rhs=v_aug[:, k, :],
                        start=(k == 0), stop=(k == jq),
                    )
                if jq % 4 == 3:
                    recip4 = small.tile([P, 4], F32, tag="recip")
                    nc.vector.reciprocal(
                        recip4,
                        o4[:, :, D:D + 1].rearrange("p a b -> p (a b)"),
                    )
                    rb = bass.AP(tensor=recip4.tensor, offset=recip4.offset,
                                 ap=[recip4.ap[0], recip4.ap[1], [0, D]])
                    nc.vector.tensor_tensor(
                        out=o_sb[:, jq - 3:jq + 1, :],
                        in0=o4[:, :, 0:D], in1=rb,
                        op=mybir.AluOpType.mult,
                    )
                    nc.sync.dma_start(
                        out=o_d[h].rearrange("(b p) d -> p b d", p=P)
                                  [:, jq - 3:jq + 1, :],
                        in_=o_sb[:, jq - 3:jq + 1, :],
                    )
                if jq == ST - 1:
                    del state[h]

            # A@V slot schedule: greedily cap matmuls per group so the PE
            # load stays even; the last head schedules greedily (no lag/cap)
            # to shrink the drain.
            def av_slots(heads_n):
                slots = {}
                load = {}
                prev = 0
                for h in range(heads_n):
                    last = h == heads_n - 1
                    for jq in range(ST):
                        lag = 0 if last else LAG
                        gk = "last" if last else "mid"
                        ready = h * NGRP + ready_group(head_kind(h), gk, jq) + lag
                        s = max(ready, prev)
                        while (not last and load.get(s, 0)
                               and load.get(s, 0) + (jq + 1) > CAP):
                            s += 1
                        load[s] = load.get(s, 0) + (jq + 1)
                        slots[(h, jq)] = s
                        prev = s
                return slots

            slot = av_slots(heads_per_core)
            tasks = [(h, jq) for h in range(heads_per_core) for jq in range(ST)]

            qT0, kT0, v_aug0, tail0 = emit_prep(0)
            state[0] = {"qT": qT0, "kT": kT0, "v_aug": v_aug0,
                        "at": atp.tile([P, STREAM], BF16, tag="at_all",
                                       name="at0"),
                        "o_sb": osbp.tile([P, ST, D], F32, tag="osb", name="osb0")}
            av_next = 0
            prefetched = {}
            for G in range(heads_per_core * NGRP):
                h, g = divmod(G, NGRP)
                if (h, g) in prefetched:
                    emit_exp(h, g, prefetched.pop((h, g)))
                elif g == NGRP - 1 and h + 1 < heads_per_core:
                    ring_last = alloc_ring(h, g)
                    ring_next = alloc_ring(h + 1, 0)
                    fill_ring(h, g, ring_last)
                    fill_ring(h + 1, 0, ring_next)
                    emit_exp(h, g, ring_last)
                    prefetched[(h + 1, 0)] = ring_next
                else:
                    emit_group(h, g)
                if h == 0 and g == 2 and tail0 is not None:
                    tail0()  # head 0's tiles 0-7 chain, after groups 0-2
                if g == PREP_G and h + 1 < heads_per_core:
                    qTn, kTn, v_augn, _ = emit_prep(h + 1)
                    state[h + 1] = {
                        "qT": qTn, "kT": kTn, "v_aug": v_augn,
                        "at": atp.tile([P, STREAM], BF16, tag="at_all",
                                       name=f"at{h + 1}"),
                        "o_sb": osbp.tile([P, ST, D], F32, tag="osb",
                                          name=f"osb{h + 1}"),
                    }
                while av_next < len(tasks) and slot[tasks[av_next]] <= G:
                    emit_av(*tasks[av_next])
                    av_next += 1
            while av_next < len(tasks):
                emit_av(*tasks[av_next])
                av_next += 1

    nc.compile()
    return nc


_NC_CACHE = {}


def _get_nc(heads_per_core=HPC):
    if heads_per_core not in _NC_CACHE:
        _NC_CACHE[heads_per_core] = build_nc(heads_per_core)
    return _NC_CACHE[heads_per_core]


def run_sharded(Q, K, V, heads_per_core=HPC, **run_kwargs):
    """Q, K, V: [HEADS-or-subset, S, D] f32 flattened over (B, H)."""
    nc = _get_nc(heads_per_core)
    n = heads_per_core
    in_maps = [
        {
            "Q": np.ascontiguousarray(Q[i * n:(i + 1) * n]),
            "K": np.ascontiguousarray(K[i * n:(i + 1) * n]),
            "V": np.ascontiguousarray(V[i * n:(i + 1) * n]),
        }
        for i in range(N_CORES)
    ]
    last_err = None
    for attempt in range(3):
        try:
            res = run_bass_kernel_spmd(nc, in_maps,
                                       core_ids=list(range(N_CORES)),
                                       **run_kwargs)
            out = np.concatenate(
                [np.asarray(res.results[i]["out"]) for i in range(N_CORES)],
                axis=0)
            return out, res
        except Exception as e:  # transient NRT_EXEC_UNIT_UNRECOVERABLE etc.
            last_err = e
            import time
            time.sleep(2.0)
    raise last_err


def kernel(Q, K, V, mask=None):
    Q = np.asarray(Q, dtype=np.float32).reshape(HEADS, S, D)
    K = np.asarray(K, dtype=np.float32).reshape(HEADS, S, D)
    V = np.asarray(V, dtype=np.float32).reshape(HEADS, S, D)
    out, _ = run_sharded(Q, K, V)
    return out.reshape(B, H, S, D)
